# revision 21
# baseline (speedup 1.0000x reference)
"""BoT tokenizer kernel for Trainium2 (Bass/Tile), 8-core data parallel.

All 25 output tokens are computed on the TensorEngine as fp8 (e4m3)
DoubleRow matmuls: the moving stream runs at 2 rows/cycle, halving PE
time vs bf16 so the PE keeps up even at its low DVFS p-state.

fp32 operands are split into 3 fp8 terms (x = a0+a1+a2, ~4 bits each) and
the matmul accumulates the 6 dominant cross products a_i*w_j (i+j <= 2),
each product pair scaled by (2^(2i-2j), 2^(2j-2i)) to keep every fp8 row
in the normal range; 2 extra rows add the bias (b0 at 2^-6, residual at
2^-8). Achieved accuracy: ~1.8e-3 l2 before output rounding.

 - single-sensor token: K = 6+2 = 8 logical rows -> Kp=4 partitions
 - fore token: K = 9*6+2 = 56 -> Kp=28;  palm: 7*6+2 = 44 -> Kp=22

DoubleRow layout: logical row k lives at partition k//2, half k%2, i.e.
lhsT tiles are [Kp, 2, B], rhs tiles [Kp, 2, D].

PSUM -> SBUF pair-copies ([128,1024], amortizing PSUM access latency)
split between ScalarE and VectorE. The output is written to HBM as bf16
(harness tolerance 2e-2 l2; this kernel ~1.9e-3) and upcast to fp32 on
the host, halving output DMA bytes vs fp32. Output DMA goes out in 3
column groups per 128-row chunk (8KB+ rows, single sync HWDGE queue),
which measured fastest. Per-core HBM write: 26.2 MB at ~380 GB/s
(16 DMA engines x ~25 GB/s); typical HW time ~93 us vs 147 us baseline.
"""

import numpy as np

FORE_IDX = [0, 1, 2, 27, 28, 32, 33, 34, 38]
PALM_IDX = [4, 29, 30, 31, 35, 36, 37]
SINGLE_IDX = [3] + list(range(5, 27))

B = 8192
D = 512
T = 25
N_CORES = 8
B_LOC = B // N_CORES          # 1024 rows per core
CHUNK = 128
N_CHUNKS = B_LOC // CHUNK     # 8
ROW = T * D                   # 12800
NS = 23

# token id for single sensor k: k=0 -> token 1 (wrist), k>=1 -> token k+2
TOK_OF_SINGLE = [1] + list(range(3, 25))
# out-tile token groups (DMA granularity)
GROUPS = [(0, 8), (8, 16), (16, 25)]
# copy batches: (t0, t1, engine) pairs + trailing single
COPY_BATCHES = [(0, 2, 'a'), (2, 4, 'v'), (4, 6, 'a'), (6, 8, 'v'),
                (8, 10, 'a'), (10, 12, 'v'), (12, 14, 'a'), (14, 16, 'v'),
                (16, 18, 'a'), (18, 20, 'v'), (20, 22, 'a'), (22, 24, 'v'),
                (24, 25, 'a')]
# cross products (i,j) of the 3-term fp8 splits kept in the contraction
PRODS = [(0, 0), (0, 1), (1, 0), (1, 1), (0, 2), (2, 0)]
KPF = (9 * 6 + 2) // 2        # 28 partitions, fore
KPP = (7 * 6 + 2) // 2        # 22 partitions, palm
KPS = (6 + 2) // 2            # 4 partitions, single
# singles packed 3 per tile at 32-partition offsets (matmul base partition
# must be 0/32/64)
S_TILES = [(a, min(a + 3, NS)) for a in range(0, NS, 3)]   # 8 tiles
S_STRIDE = 32

_prog_cache = {}


def _k_of_tok(t):
    return 0 if t == 1 else t - 2


def _build_program():
    import concourse.bacc as bacc
    import concourse.mybir as mybir
    import concourse.tile as tile
    from concourse.bass import ts

    f32 = mybir.dt.float32
    bf16 = mybir.dt.bfloat16
    fp8 = mybir.dt.float8e4
    dr = mybir.MatmulPerfMode.DoubleRow
    nc = bacc.Bacc("TRN2", target_bir_lowering=False, debug=False,
                   num_devices=N_CORES)

    lf_d = nc.dram_tensor("lf", [KPF, 2, B_LOC], fp8, kind="ExternalInput")
    lp_d = nc.dram_tensor("lp", [KPP, 2, B_LOC], fp8, kind="ExternalInput")
    rf_d = nc.dram_tensor("rf", [KPF, 2, D], fp8, kind="ExternalInput")
    rp_d = nc.dram_tensor("rp", [KPP, 2, D], fp8, kind="ExternalInput")
    # singles: one dram tensor per 32-partition slot j, all 8 sensor
    # groups side by side along the free dim
    NG = len(S_TILES)             # 8
    ls_d = [nc.dram_tensor(f"ls{j}", [KPS, 2, NG * B_LOC], fp8,
                           kind="ExternalInput") for j in range(3)]
    rs_d = [nc.dram_tensor(f"rs{j}", [KPS, 2, NG * D], fp8,
                           kind="ExternalInput") for j in range(3)]
    out_d = nc.dram_tensor("out", [B_LOC, ROW], bf16, kind="ExternalOutput")

    with tile.TileContext(nc) as tc:
        with (
            tc.tile_pool(name="cst", bufs=1) as cst,
            tc.tile_pool(name="op", bufs=4) as op,
            tc.tile_pool(name="pp", bufs=4, space="PSUM") as pp,
        ):
            lf_s = cst.tile([KPF, 2, B_LOC], fp8)
            nc.sync.dma_start(out=lf_s[:], in_=lf_d[:])
            rf_s = cst.tile([KPF, 2, D], fp8)
            nc.sync.dma_start(out=rf_s[:], in_=rf_d[:])
            lp_s = cst.tile([KPP, 2, B_LOC], fp8)
            nc.sync.dma_start(out=lp_s[:], in_=lp_d[:])
            rp_s = cst.tile([KPP, 2, D], fp8)
            nc.sync.dma_start(out=rp_s[:], in_=rp_d[:])
            ls_big = cst.tile([96, 2, NG * B_LOC], fp8)
            rs_big = cst.tile([96, 2, NG * D], fp8)
            for j in range(3):
                o = S_STRIDE * j
                nc.sync.dma_start(out=ls_big[o:o + KPS, :, :], in_=ls_d[j][:])
                nc.sync.dma_start(out=rs_big[o:o + KPS, :, :], in_=rs_d[j][:])

            def lhs_rhs(t, c):
                if t == 0:
                    return lf_s[:, :, ts(c, CHUNK)], rf_s[:]
                if t == 2:
                    return lp_s[:, :, ts(c, CHUNK)], rp_s[:]
                k = _k_of_tok(t)
                i = k // 3
                off = S_STRIDE * (k % 3)
                cb = i * B_LOC + c * CHUNK
                return (ls_big[off:off + KPS, :, cb:cb + CHUNK],
                        rs_big[off:off + KPS, :, i * D:(i + 1) * D])

            for c in range(N_CHUNKS):
                o_t = {}
                for gi, (t0, t1) in enumerate(GROUPS):
                    o_t[gi] = op.tile([CHUNK, (t1 - t0) * D], bf16,
                                      tag=f"out{gi}", name=f"out{gi}")
                for (b0, b1, eng) in COPY_BATCHES:
                    p_t = pp.tile([CHUNK, 2 * D], f32)
                    for t in range(b0, b1):
                        lhsT, rhs = lhs_rhs(t, c)
                        nc.tensor.matmul(p_t[:, ts(t - b0, D)], lhsT, rhs,
                                         start=True, stop=True, perf_mode=dr)
                    gi = next(i for i, (t0, t1) in enumerate(GROUPS)
                              if t0 <= b0 < t1)
                    g0 = GROUPS[gi][0]
                    dst = o_t[gi][:, (b0 - g0) * D:(b1 - g0) * D]
                    src = p_t[:, 0:(b1 - b0) * D]
                    if eng == 'a':
                        nc.scalar.copy(dst, src)
                    else:
                        nc.vector.tensor_copy(dst, src)
                    if b1 in (8, 16, 25):
                        gi = {8: 0, 16: 1, 25: 2}[b1]
                        t0, t1 = GROUPS[gi]
                        nc.sync.dma_start(
                            out=out_d[ts(c, CHUNK), t0 * D:t1 * D],
                            in_=o_t[gi][:])

    nc.compile()
    return nc


def _split3_f8(v):
    """fp32 -> 3 fp8(e4m3) terms: v ~= a0+a1+a2."""
    import ml_dtypes
    f8 = ml_dtypes.float8_e4m3
    v = np.asarray(v, np.float32)
    a0 = v.astype(f8)
    r1 = v - a0.astype(np.float32)
    a1 = r1.astype(f8)
    r2 = r1 - a1.astype(np.float32)
    a2 = r2.astype(f8)
    return a0, a1, a2


def _build_pair(xcols, wrows, bias):
    """lhs/rhs row stacks for one token's DoubleRow matmul.

    xcols: [B, F] fp32 (features), wrows: [F, D] fp32, bias: [D] fp32.
    Returns lhs [Kp, 2, B] fp8, rhs [Kp, 2, D] fp8 with K = 6F+2."""
    import ml_dtypes
    f8 = ml_dtypes.float8_e4m3
    Bn, F = xcols.shape
    K = 6 * F + 2
    lhs = np.zeros((K, Bn), dtype=f8)
    rhs = np.zeros((K, D), dtype=f8)
    ax = _split3_f8(xcols)               # each [B, F]
    aw = _split3_f8(wrows)               # each [F, D]
    for f in range(F):
        for p, (i, j) in enumerate(PRODS):
            k = f * 6 + p
            lam = np.float32(2.0 ** (2 * i - 2 * j))
            lhs[k] = (ax[i][:, f].astype(np.float32) * lam).astype(f8)
            rhs[k] = (aw[j][f].astype(np.float32) / lam).astype(f8)
    b0 = (bias * 2.0 ** 6).astype(f8)
    rb = bias - b0.astype(np.float32) * 2.0 ** -6
    b1 = (rb * 2.0 ** 8).astype(f8)
    lhs[6 * F] = np.float32(2.0 ** -6)
    rhs[6 * F] = b0
    lhs[6 * F + 1] = np.float32(2.0 ** -8)
    rhs[6 * F + 1] = b1
    # DoubleRow packing: logical row k -> (partition k//2, half k%2)
    return (lhs.reshape(K // 2, 2, Bn), rhs.reshape(K // 2, 2, D))


def _host_prep(x, Wf, bf_, Wp, bp, Ws, bs):
    import ml_dtypes
    f8 = ml_dtypes.float8_e4m3

    lf, rf = _build_pair(x[:, FORE_IDX], np.asarray(Wf.T), bf_)
    lp, rp = _build_pair(x[:, PALM_IDX], np.asarray(Wp.T), bp)

    xs = x[:, SINGLE_IDX]                # [B, 23]
    NG = len(S_TILES)
    ls_all = np.zeros((3, KPS, 2, NG, B), dtype=f8)
    rs_all = np.zeros((3, KPS, 2, NG, D), dtype=f8)
    for k in range(NS):
        i, j = k // 3, k % 3
        lsk, rsk = _build_pair(xs[:, k:k + 1], Ws[k:k + 1], bs[k])
        ls_all[j, :, :, i] = lsk
        rs_all[j, :, :, i] = rsk
    return lf, rf, lp, rp, ls_all, rs_all


def kernel(x, Wf, bf, Wp, bp, Ws, bs, _trace=False, _spmd_kwargs=None):
    from concourse.bass_utils import run_bass_kernel_spmd

    x = np.asarray(x, np.float32)
    lf, rf, lp, rp, ls_all, rs_all = _host_prep(
        x, np.asarray(Wf, np.float32), np.asarray(bf, np.float32),
        np.asarray(Wp, np.float32), np.asarray(bp, np.float32),
        np.asarray(Ws, np.float32), np.asarray(bs, np.float32))

    if "nc" not in _prog_cache:
        _prog_cache["nc"] = _build_program()
    nc = _prog_cache["nc"]

    in_maps = []
    for i in range(N_CORES):
        sl = slice(i * B_LOC, (i + 1) * B_LOC)
        m = {
            "lf": np.ascontiguousarray(lf[:, :, sl]),
            "lp": np.ascontiguousarray(lp[:, :, sl]),
            "rf": rf,
            "rp": rp,
        }
        ng = ls_all.shape[3]
        for j in range(3):
            m[f"ls{j}"] = np.ascontiguousarray(
                ls_all[j][:, :, :, sl]).reshape(KPS, 2, ng * B_LOC)
            m[f"rs{j}"] = np.ascontiguousarray(rs_all[j]).reshape(
                KPS, 2, ng * D)
        in_maps.append(m)

    kwargs = dict(_spmd_kwargs or {})
    res = run_bass_kernel_spmd(nc, in_maps, core_ids=list(range(N_CORES)),
                               trace=_trace, **kwargs)
    out = np.concatenate([np.asarray(r["out"]) for r in res.results], axis=0)
    if _trace:
        kernel.last_results = res
    return out.astype(np.float32).reshape(B, T, D)
